# revision 37
# baseline (speedup 1.0000x reference)
"""Trainium2 Bass kernel for nn_Loss_dict_50646254354805 (NeRF-style loss).

Self-contained: accepts FULL inputs, shards across 8 NeuronCores (rays for
the per-ray losses, samples for the hash loss), runs one SPMD Bass module,
host-sums the 8 partial scalars.

Inter-loss algorithm: the reference's blur_step_function + sorted_interp_quad
is equivalent (exactly, in real arithmetic) to evaluating
    cdf(x) = 0.5 * sum_j r_j * relu(x - e_j)^2
where e_j are the 98 blur events (sdist -+ pw) and r_j the signed slope
deltas (+-radio). With prefix sums P0/P1/P2 of r*{1,e,e^2} in event order,
cdf(x) = 0.5*(x^2 P0_k - 2x P1_k + P2_k) at k = rank of x among events.
The kernel merges int16-quantized keys (queries+events) bitonically, scatters
the f32 payloads r/re/re^2 into event slots of the merged domain (as u16
pairs with doubled indices), prefix-scans them (P at each query slot = P_k),
compacts P0/P1/P2 at query slots, and evaluates the quadratic per query.
Quantization only affects event/query interleaving for pairs closer than
~1.3e-4, which is far inside the tolerance.
"""
import numpy as np

import concourse.bass as bass
import concourse.mybir as mybir
import concourse.tile as tile
from concourse import bacc
from concourse.bass_utils import run_bass_kernel_spmd

dt = mybir.dt
Alu = mybir.AluOpType
AX = mybir.AxisListType
Act = mybir.ActivationFunctionType
P = 128

# problem constants
PULSE = (0.01, 0.005)
W_RGB, W_INTER, W_DIST, W_HASH = 1.0, 1.0, 0.01, 0.1
NUM_SEGMENTS = 65536
R, N = 4096, 48
M = R * N
N_CORES = 8
RPC = R // N_CORES            # rays per core (512)
NBLK = RPC // P               # ray tiles per core (4)
MPC = M // N_CORES            # hash samples per core (24576)
HALO = 64                     # hash run halo
HROW = MPC // P               # hash samples per partition (192)
HCOLS = HROW + HALO + 1       # loaded cols per partition (257)
HSLICE = HALO + MPC + HALO    # per-core hash slice length (24704)

# key quantization: key = int16(v*QSCALE + QOFF)*4 + tag, tags em=0 ep=1 q=2
QSCALE = 8000.0
QOFF = 100.0
BIGK = 32767                  # pad key (sorts after all real keys)

# per-level geometry
LVL = {0: dict(X=257, n2=512), 1: dict(X=97, n2=256)}
for _L in LVL.values():
    _L["LW"] = ((_L["X"] + 98 + 1 + 7) // 8) * 8        # 360 / 200
    _L["XW"] = ((_L["X"] + 7) // 8) * 8                 # 264 / 104


def _ts_int(eng, out, in0, imm1, op0, imm2=None, op1=None):
    """tensor_scalar with int32 immediates (for int16 ops)."""
    ins_ = [eng.lower_ap(in0), mybir.ImmediateValue(dtype=dt.int32, value=int(imm1))]
    kw = dict(op0=op0)
    if imm2 is not None:
        ins_.append(mybir.ImmediateValue(dtype=dt.int32, value=int(imm2)))
        kw["op1"] = op1
    return eng.add_instruction(mybir.InstTensorScalarPtr(
        name=eng.bass.get_next_instruction_name(),
        ins=ins_, outs=[eng.lower_ap(out)], **kw))


def _bcast_row(nc, dst_ap, src_ap, n, eng=None):
    eng = eng or nc.sync
    eng.dma_start(dst_ap, src_ap[:, 0:n])


def _blk(ap, n2):
    return ap.rearrange("p (b n) -> p b n", b=NBLK)


def _bitonic_merge(eng, bufa, bufb, width, descending, nblk=NBLK, alt_eng=None):
    """Ping-pong bitonic merge over [P, nblk*width] int16 tiles.

    If alt_eng is given, the two compare ops of each stage run on different
    engines (min on eng, max on alt_eng). Returns (result, scratch)."""
    cur, nxt = bufa, bufb
    d = width // 2
    while d >= 1:
        c3 = cur[:].rearrange("p (c td) -> p c td", td=2 * d)
        n3 = nxt[:].rearrange("p (c td) -> p c td", td=2 * d)
        lo_in, hi_in = c3[:, :, 0:d], c3[:, :, d:2 * d]
        e2 = alt_eng or eng
        if descending:
            eng.tensor_tensor(n3[:, :, 0:d], lo_in, hi_in, Alu.max)
            e2.tensor_tensor(n3[:, :, d:2 * d], lo_in, hi_in, Alu.min)
        else:
            eng.tensor_tensor(n3[:, :, 0:d], lo_in, hi_in, Alu.min)
            e2.tensor_tensor(n3[:, :, d:2 * d], lo_in, hi_in, Alu.max)
        cur, nxt = nxt, cur
        d //= 2
    return cur, nxt


def _bitonic_merge_ap(eng, apa, apb, width, descending, alt_eng=None):
    """AP-based ping-pong bitonic merge. Returns the result AP."""
    cur, nxt = apa, apb
    d = width // 2
    while d >= 1:
        c3 = cur.rearrange("p (c td) -> p c td", td=2 * d)
        n3 = nxt.rearrange("p (c td) -> p c td", td=2 * d)
        lo_in, hi_in = c3[:, :, 0:d], c3[:, :, d:2 * d]
        e2 = alt_eng or eng
        if descending:
            eng.tensor_tensor(n3[:, :, 0:d], lo_in, hi_in, Alu.max)
            e2.tensor_tensor(n3[:, :, d:2 * d], lo_in, hi_in, Alu.min)
        else:
            eng.tensor_tensor(n3[:, :, 0:d], lo_in, hi_in, Alu.min)
            e2.tensor_tensor(n3[:, :, d:2 * d], lo_in, hi_in, Alu.max)
        cur, nxt = nxt, cur
        d //= 2
    return cur, nxt


def _emit_level(nc, tc, wpool, pool, lvl, ekeys, radio, emc, epc, x_ap,
                pwt_ap, inter_acc, aps, neg05):
    """Inter loss for one prop level via the P0/P1/P2 prefix-sum scheme.

    Big merge / merged-domain buffers come from the shared work pool (both
    levels use the same allocations, sliced to this level's width)."""
    L = LVL[lvl]
    X, n2, LW, XW = L["X"], L["n2"], L["LW"], L["XW"]
    SL = NBLK * n2               # merge width total
    NL = NBLK * LW               # compact merged width
    NX = NBLK * XW               # compact query width
    pw = PULSE[lvl]
    SLM = NBLK * 512             # max merge width (level 0)
    NLM = NBLK * 360             # max compact width (level 0)

    def wtile(tag, width, dtype, maxw):
        t = wpool.tile([P, maxw], dtype, tag=tag, name=tag)
        return t, t[:][:, 0:width]

    def ltile(tag, width, dtype):
        t = pool.tile([P, width], dtype, tag=tag, name=tag)
        return t, t[:][:, 0:width]

    def blkL(ap):
        return ap.rearrange("p (b n) -> p b n", b=NBLK)

    # ---------- consts ----------
    iota_loc16 = aps["iotash"]
    iota1_16 = pool.tile([P, NL], dt.int16, tag="iota1_16")
    _bcast_row(nc, iota1_16[:], aps[f"c_iota1_l{lvl}"], NL)
    maski = pool.tile([P, NL], dt.int16, tag="maski")
    _bcast_row(nc, maski[:], aps[f"c_maski_l{lvl}"], NL)
    maskf = pool.tile([P, NL], dt.float32, tag="maskf")
    _bcast_row(nc, maskf[:], aps[f"c_maskf_l{lvl}"], NL)

    # ---------- inputs ----------
    xt = pool.tile([P, NBLK * X], dt.float32, tag="xt")
    nc.sync.dma_start(_blk(xt[:], X), x_ap.rearrange("(p b) x -> p b x", b=NBLK))
    pwt = pool.tile([P, NBLK * (X - 1)], dt.float32, tag="pwt")
    nc.sync.dma_start(_blk(pwt[:], X - 1), pwt_ap.rearrange("(p b) x -> p b x", b=NBLK))

    # ---------- query keys ----------
    qsf = pool.tile([P, NBLK * X], dt.float32, tag="qsf")
    nc.scalar.activation(qsf[:], xt[:], Act.Copy, bias=QOFF, scale=QSCALE)
    qki = pool.tile([P, NBLK * X], dt.int16, tag="qki")
    nc.vector.tensor_copy(qki[:], qsf[:])
    _ts_int(nc.vector, qki[:], qki[:], 4, Alu.mult, 2, Alu.add)

    # ---------- big merge: queries + events, ascending ----------
    B0t, B0a = ltile("big0", SL, dt.int16)
    B1t, B1a = ltile("big1", SL, dt.int16)
    nc.gpsimd.memset(B0a, BIGK)
    b03 = _blk(B0a, n2)
    nc.gpsimd.tensor_copy(b03[:, :, 0:X], _blk(qki[:], X))
    # event block (descending, from the combined event merge) at the tail
    ek_l = ekeys[:][:, 512 * lvl:512 * (lvl + 1)].rearrange(
        "p (b n) -> p b n", b=NBLK)
    nc.gpsimd.tensor_copy(b03[:, :, n2 - 128:n2], ek_l)
    SMa, _ = _bitonic_merge_ap(nc.vector, B0a, B1a, n2, descending=False)
    mS = _blk(SMa, n2)[:, :, 0:LW]     # compact strided view of merged keys

    # ---------- tags (i32 bitwise over key pairs, then i16 compares) ----------
    tg_t, tagb = ltile("tagb", SL, dt.int16)
    _ts_int(nc.vector, tagb.bitcast(dt.int32), SMa.bitcast(dt.int32),
            0x00030003, Alu.bitwise_and)
    tagS = _blk(tagb, n2)[:, :, 0:LW]
    em_t, em_f = ltile("em_f", NL, dt.int16)
    _ts_int(nc.vector, em_f, tagS, 0, Alu.is_equal)
    ev_t, ev_f = ltile("ev_f", NL, dt.int16)
    _ts_int(nc.vector, ev_f, tagS, 1, Alu.is_le)
    qf_t, qf = ltile("qf", NL, dt.int16)
    _ts_int(nc.vector, qf, tagS, 2, Alu.is_equal)

    # ---------- counts ----------
    C_t, C = ltile("C", NL, dt.int16)
    nc.vector.tensor_tensor_scan(C, maski[:], ev_f, 0.0, Alu.mult, Alu.add)
    Cm_t, Cm = ltile("Cm", NL, dt.int16)
    nc.vector.tensor_tensor_scan(Cm, maski[:], em_f, 0.0, Alu.mult, Alu.add)

    # ---------- event positions ----------
    ix_t, idx16 = ltile("idx16", NL, dt.int16)
    tt_t, t16 = ltile("t16", NL, dt.int16)
    pos_m = pool.tile([P, NBLK * 64], dt.int16, tag="pos_m")
    pos_p = pool.tile([P, NBLK * 64], dt.int16, tag="pos_p")
    # em side: idx = Cm*em_f - 1
    nc.vector.tensor_tensor(idx16, Cm, em_f, Alu.mult)
    _ts_int(nc.vector, idx16, idx16, -1, Alu.add)
    for b in range(NBLK):
        nc.gpsimd.local_scatter(pos_m[:, b * 64:(b + 1) * 64],
                                iota_loc16[:, b * 512:b * 512 + LW],
                                idx16[:, b * LW:b * LW + LW], channels=P,
                                num_elems=64, num_idxs=LW)
    # ep side: idx = (C-Cm)*ep_f - 1 ; ep_f = ev_f - em_f
    nc.vector.tensor_tensor(t16, C, Cm, Alu.subtract)
    nc.vector.tensor_tensor(idx16, ev_f, em_f, Alu.subtract)
    nc.vector.tensor_tensor(idx16, idx16, t16, Alu.mult)
    _ts_int(nc.vector, idx16, idx16, -1, Alu.add)
    for b in range(NBLK):
        nc.gpsimd.local_scatter(pos_p[:, b * 64:(b + 1) * 64],
                                iota_loc16[:, b * 512:b * 512 + LW],
                                idx16[:, b * LW:b * LW + LW], channels=P,
                                num_elems=64, num_idxs=LW)

    # ---------- doubled scatter targets (u16-pair trick) ----------
    tgt = pool.tile([P, NBLK * 128], dt.int16, tag="tgt")
    t3 = _blk(tgt[:], 128)
    nc.vector.tensor_copy(t3[:, :, 0:49], _blk(pos_m[:], 64)[:, :, 0:49])
    nc.vector.tensor_copy(t3[:, :, 49:98], _blk(pos_p[:], 64)[:, :, 0:49])
    nc.gpsimd.memset(t3[:, :, 98:128], -1)
    tgt2 = pool.tile([P, NBLK * 256], dt.int16, tag="tgt2")
    t2v = tgt2[:].rearrange("p (n two) -> p n two", two=2)
    _ts_int(nc.vector, t2v[:, :, 0], tgt[:], 2, Alu.mult)
    _ts_int(nc.vector, t2v[:, :, 1], tgt[:], 2, Alu.mult, 1, Alu.add)

    # ---------- payloads in cat layout [em(49) | ep(49) | pad] ----------
    rem = pool.tile([P, NBLK * 49], dt.float32, tag="rem")
    rep = pool.tile([P, NBLK * 49], dt.float32, tag="rep")
    nc.gpsimd.tensor_tensor(rem[:], radio[:], emc[:], Alu.mult)
    nc.gpsimd.tensor_tensor(rep[:], radio[:], epc[:], Alu.mult)
    rem2 = pool.tile([P, NBLK * 49], dt.float32, tag="rem2")
    rep2 = pool.tile([P, NBLK * 49], dt.float32, tag="rep2")
    nc.gpsimd.tensor_tensor(rem2[:], rem[:], emc[:], Alu.mult)
    nc.gpsimd.tensor_tensor(rep2[:], rep[:], epc[:], Alu.mult)
    cats = {}
    for nm, plus, minus in (("r", radio, radio), ("re", rem, rep),
                            ("re2", rem2, rep2)):
        c = pool.tile([P, NBLK * 128], dt.float32, tag=f"cat_{nm}")
        c3 = _blk(c[:], 128)
        nc.gpsimd.memset(c3[:, :, 98:128], 0.0)
        nc.gpsimd.tensor_copy(c3[:, :, 0:49], _blk(plus[:], 49))
        nc.gpsimd.tensor_scalar(c3[:, :, 49:98], _blk(minus[:], 49), -1.0,
                                None, Alu.mult)
        cats[nm] = c

    # ---------- scatter payloads into merged domain (zeroes dest) ----------
    merged = {}
    for nm in ("r", "re", "re2"):
        _, m = wtile(f"m_{nm}", NL, dt.float32, NLM)
        mu = m.bitcast(dt.uint16)
        cu = cats[nm][:].bitcast(dt.uint16)
        for b in range(NBLK):
            nc.gpsimd.local_scatter(mu[:, b * 2 * LW:(b + 1) * 2 * LW],
                                    cu[:, b * 256:(b + 1) * 256],
                                    tgt2[:, b * 256:(b + 1) * 256], channels=P,
                                    num_elems=2 * LW, num_idxs=256)
        merged[nm] = m

    # ---------- P scans (in place over the scattered payloads) ----------
    P0, P1, P2 = merged["r"], merged["re"], merged["re2"]
    nc.vector.tensor_tensor_scan(P0, maskf[:], P0, 0.0, Alu.mult, Alu.add)
    nc.vector.tensor_tensor_scan(P1, maskf[:], P1, 0.0, Alu.mult, Alu.add)
    nc.vector.tensor_tensor_scan(P2, maskf[:], P2, 0.0, Alu.mult, Alu.add)

    # ---------- compact P's at query slots ----------
    # idxq = (iota1 - C)*qf - 1  (block-local 0-based query index, -1 elsewhere)
    nc.vector.tensor_tensor(t16, iota1_16[:], C, Alu.subtract)
    nc.vector.tensor_tensor(t16, t16, qf, Alu.mult)
    _ts_int(nc.vector, t16, t16, -1, Alu.add)
    _, idxq2 = wtile("idxq2", 2 * NL, dt.int16, 2 * NLM)
    q2v = idxq2.rearrange("p (n two) -> p n two", two=2)
    _ts_int(nc.vector, q2v[:, :, 0], t16, 2, Alu.mult)
    _ts_int(nc.vector, q2v[:, :, 1], t16, 2, Alu.mult, 1, Alu.add)
    comp = {}
    for nm, src in (("P0", P0), ("P1", P1), ("P2", P2)):
        c = pool.tile([P, NX], dt.float32, tag=f"c_{nm}", name=f"c_{nm}")
        cu = c[:].bitcast(dt.uint16)
        su = src.bitcast(dt.uint16)
        for b in range(NBLK):
            nc.gpsimd.local_scatter(cu[:, b * 2 * XW:(b + 1) * 2 * XW],
                                    su[:, b * 2 * LW:(b + 1) * 2 * LW],
                                    idxq2[:, b * 2 * LW:(b + 1) * 2 * LW],
                                    channels=P, num_elems=2 * XW,
                                    num_idxs=2 * LW)
        comp[nm] = c

    # ---------- query-domain cdf + loss tail (deferred; aggressive reuse) ----
    def tail():
        emit_tail(nc, pool, lvl, comp, xt, pwt, inter_acc, neg05)
    return tail


def emit_tail(nc, pool, lvl, comp, xt, pwt, inter_acc, neg05):
    L = LVL[lvl]
    X, XW = L["X"], L["XW"]
    NQ = NBLK * X
    NW = NBLK * (X - 1)
    xc = pool.tile([P, NQ], dt.float32, tag="xc")
    nc.vector.tensor_scalar(xc[:], xt[:], -0.5, None, Alu.add)
    xx = pool.tile([P, NQ], dt.float32, tag="xx")
    nc.scalar.activation(xx[:], xt[:], Act.Square, bias=neg05[:])
    t1 = pool.tile([P, NQ], dt.float32, tag="t1q")
    t2q = pool.tile([P, NQ], dt.float32, tag="t2q")
    x3 = _blk(xx[:], X)
    c3q = _blk(xc[:], X)
    cP0 = _blk(comp["P0"][:], XW)[:, :, 0:X]
    cP1 = _blk(comp["P1"][:], XW)[:, :, 0:X]
    cP2 = _blk(comp["P2"][:], XW)[:, :, 0:X]
    nc.gpsimd.tensor_tensor(_blk(t1[:], X), x3, cP0, Alu.mult)
    nc.gpsimd.tensor_tensor(_blk(t2q[:], X), c3q, cP1, Alu.mult)
    nc.vector.scalar_tensor_tensor(_blk(t1[:], X), _blk(t2q[:], X), -2.0,
                                   _blk(t1[:], X), Alu.mult, Alu.add)
    cdf = pool.tile([P, NQ], dt.float32, tag="xx")      # xx dead after t1
    nc.gpsimd.tensor_tensor(_blk(cdf[:], X), _blk(t1[:], X), cP2, Alu.add)
    # d = cdf[i+1]-cdf[i]  (= 2*ws);  loss term = 0.25*relu(d-2*pwt)^2*den
    dq = pool.tile([P, NQ], dt.float32, tag="xc")       # xc dead after t2q
    cd3 = _blk(cdf[:], X)
    nc.gpsimd.tensor_tensor(_blk(dq[:][:, 0:NW], X - 1), cd3[:, :, 1:X],
                            cd3[:, :, 0:X - 1], Alu.subtract)
    den = pool.tile([P, NQ], dt.float32, tag="t2q")     # t2q dead after stt
    nc.gpsimd.tensor_scalar(den[:][:, 0:NW], pwt[:], 1e-5, None, Alu.add)
    nc.vector.reciprocal(den[:][:, 0:NW], den[:][:, 0:NW])
    z = pool.tile([P, NQ], dt.float32, tag="t1q")       # t1 dead after cdf
    nc.vector.scalar_tensor_tensor(z[:][:, 0:NW], pwt[:], -2.0,
                                   dq[:][:, 0:NW], Alu.mult, Alu.add)
    rz = pool.tile([P, NQ], dt.float32, tag="qsf")      # qsf dead after qki
    nc.scalar.activation(rz[:][:, 0:NW], z[:][:, 0:NW], Act.Relu)
    nc.gpsimd.tensor_tensor(z[:][:, 0:NW], z[:][:, 0:NW], rz[:][:, 0:NW],
                            Alu.mult)
    nc.gpsimd.tensor_tensor(z[:][:, 0:NW], z[:][:, 0:NW], den[:][:, 0:NW],
                            Alu.mult)
    part = pool.tile([P, 1], dt.float32, tag="part")
    nc.vector.tensor_reduce(part[:], _blk(z[:][:, 0:NW], X - 1), AX.XY, Alu.add)
    nc.vector.tensor_scalar(inter_acc[:], part[:], 0.25 / (R * (X - 1)), None,
                            Alu.mult)


def build_module(parts=("rgb", "dist", "hash", "l0", "l1")):
    nc = bacc.Bacc("TRN2", target_bir_lowering=False, debug=False,
                   enable_asserts=False, num_devices=N_CORES)
    aps = {}

    def din(name, shape, dtype=dt.float32):
        aps[name] = nc.dram_tensor(name, shape, dtype, kind="ExternalInput").ap()
    din("pd", [RPC, 3]); din("gt", [RPC, 3])
    din("sd", [RPC, 49]); din("rw", [RPC, 48])
    din("ps0", [RPC, 257]); din("pw0", [RPC, 256])
    din("ps1", [RPC, 97]); din("pw1", [RPC, 96])
    din("hi0", [HSLICE], dt.int32); din("he0", [HSLICE * 2])
    din("hi1", [HSLICE], dt.int32); din("he1", [HSLICE * 2])
    for lvl, L in LVL.items():
        din(f"c_iota16_l{lvl}", [P, NBLK * L["n2"]], dt.int16)
        din(f"c_iota1_l{lvl}", [P, NBLK * L["LW"]], dt.int16)
        din(f"c_maski_l{lvl}", [P, NBLK * L["LW"]], dt.int16)
        din(f"c_maskf_l{lvl}", [P, NBLK * L["LW"]])
    din("c_mask48", [P, NBLK * 48]); din("c_ones", [P, HCOLS])
    out_ap = nc.dram_tensor("out", [1, 1], dt.float32, kind="ExternalOutput").ap()

    with tile.TileContext(nc) as tc:
        _emit(nc, tc, aps, out_ap, parts)
    nc.compile()
    return nc


def _emit(nc, tc, aps, out_ap, parts=("rgb", "dist", "hash", "l0", "l1")):
    import contextlib
    with contextlib.ExitStack() as ctx:
        cpool = ctx.enter_context(tc.tile_pool(name="consts", bufs=1))
        mask48 = cpool.tile([P, NBLK * 48], dt.float32, tag="mask48")
        _bcast_row(nc, mask48[:], aps["c_mask48"], NBLK * 48)
        ones_h = cpool.tile([P, HCOLS], dt.float32, tag="ones_h")
        _bcast_row(nc, ones_h[:], aps["c_ones"], HCOLS)
        neg05 = cpool.tile([P, 1], dt.float32, tag="neg05")
        nc.gpsimd.memset(neg05[:], -0.5)
        iotash = cpool.tile([P, NBLK * 512], dt.int16, tag="iotash")
        _bcast_row(nc, iotash[:], aps["c_iota16_l0"], NBLK * 512)
        aps["iotash"] = iotash[:]

        accs = {}
        for name in ("rgb", "inter", "inter1", "p1", "p2", "hash"):
            a = cpool.tile([P, 1], dt.float32, tag=f"acc_{name}")
            accs[name] = a
        for a in accs.values():
            nc.vector.memset(a[:], 0.0)

        # ---------- shared render tables + radio + dist ----------
        spool = ctx.enter_context(tc.tile_pool(name="shared", bufs=1))
        s_sh = spool.tile([P, NBLK * 49], dt.float32, tag="s_sh")
        nc.sync.dma_start(_blk(s_sh[:], 49),
                          aps["sd"].rearrange("(p b) x -> p b x", b=NBLK))
        radios = {l: spool.tile([P, NBLK * 49], dt.float32, tag=f"radio{l}",
                                name=f"radio{l}") for l in (0, 1)}
        # centered event values per level (emc = s-0.5-pw, epc = s-0.5+pw)
        emcs = {l: spool.tile([P, NBLK * 49], dt.float32, tag=f"emc{l}",
                               name=f"emc{l}") for l in (0, 1)}
        epcs = {l: spool.tile([P, NBLK * 49], dt.float32, tag=f"epc{l}",
                               name=f"epc{l}") for l in (0, 1)}
        # combined event-merge keys for both levels: [P, 8*128] int16,
        # blocks 0-3 = level0, blocks 4-7 = level1, each merged descending
        ekeys = spool.tile([P, 2 * NBLK * 128], dt.int16, tag="ekeys")

        with tc.tile_pool(name="setup", bufs=1) as pool:
            rw_sh = pool.tile([P, NBLK * 48], dt.float32, tag="rw_sh")
            nc.sync.dma_start(_blk(rw_sh[:], 48),
                              aps["rw"].rearrange("(p b) x -> p b x", b=NBLK))
            s3 = _blk(s_sh[:], 49)
            ds = pool.tile([P, NBLK * 48], dt.float32, tag="ds")
            nc.vector.tensor_tensor(_blk(ds[:], 48), s3[:, :, 1:49],
                                    s3[:, :, 0:48], Alu.subtract)
            dse = pool.tile([P, NBLK * 48], dt.float32, tag="dse")
            nc.vector.tensor_scalar(dse[:], ds[:], 1e-8, None, Alu.add)
            wnorm = pool.tile([P, NBLK * 48], dt.float32, tag="wnorm")
            nc.vector.reciprocal(dse[:], dse[:])
            nc.vector.tensor_tensor(wnorm[:], rw_sh[:], dse[:], Alu.mult)
            wnp = pool.tile([P, NBLK * 50], dt.float32, tag="wnp")
            nc.vector.memset(wnp[:], 0.0)
            nc.vector.tensor_copy(_blk(wnp[:], 50)[:, :, 1:49], _blk(wnorm[:], 48))
            diff = pool.tile([P, NBLK * 49], dt.float32, tag="diff")
            wnp3 = _blk(wnp[:], 50)
            nc.vector.tensor_tensor(_blk(diff[:], 49), wnp3[:, :, 1:50],
                                    wnp3[:, :, 0:49], Alu.subtract)
            for lvl in (0, 1):
                pw = PULSE[lvl]
                nc.vector.tensor_scalar(radios[lvl][:], diff[:],
                                        1.0 / (2 * pw), None, Alu.mult)
                nc.gpsimd.tensor_scalar(emcs[lvl][:], s_sh[:], -(0.5 + pw),
                                        None, Alu.add)
                nc.gpsimd.tensor_scalar(epcs[lvl][:], s_sh[:], -(0.5 - pw),
                                        None, Alu.add)

            # --- event keys (both levels) + combined descending merge ---
            tq = pool.tile([P, NBLK * 49], dt.float32, tag="tq")
            nc.scalar.activation(tq[:], s_sh[:], Act.Copy, bias=QOFF,
                                 scale=QSCALE)
            ek_a = pool.tile([P, 2 * NBLK * 128], dt.int16, tag="ek_a")
            ek_b = pool.tile([P, 2 * NBLK * 128], dt.int16, tag="ek_b")
            nc.gpsimd.memset(ek_a[:], BIGK)
            ef = pool.tile([P, NBLK * 49], dt.float32, tag="ef")
            ei = pool.tile([P, NBLK * 49], dt.int16, tag="ei")
            for lvl in (0, 1):
                pwS = PULSE[lvl] * QSCALE
                eka_l = ek_a[:][:, 512 * lvl:512 * (lvl + 1)].rearrange(
                    "p (b n) -> p b n", b=NBLK)
                # em keys ascending at [0:49]
                nc.vector.tensor_scalar(ef[:], tq[:], -pwS, None, Alu.add)
                nc.vector.tensor_copy(ei[:], ef[:])
                _ts_int(nc.vector, ei[:], ei[:], 4, Alu.mult, 0, Alu.add)
                nc.vector.tensor_copy(eka_l[:, :, 0:49], _blk(ei[:], 49))
                # ep keys reversed (descending) at [79:128]
                nc.vector.tensor_scalar(ef[:], tq[:], pwS, None, Alu.add)
                nc.vector.tensor_copy(ei[:], ef[:])
                _ts_int(nc.vector, ei[:], ei[:], 4, Alu.mult, 1, Alu.add)
                nc.vector.tensor_copy(eka_l[:, :, 79:128],
                                      _blk(ei[:], 49)[:, :, ::-1])
            ekm, _ = _bitonic_merge(nc.vector, ek_a, ek_b, 128,
                                    descending=True, nblk=2 * NBLK)
            nc.vector.tensor_copy(ekeys[:], ekm[:])

            # --- distortion ---
            mid = pool.tile([P, NBLK * 48], dt.float32, tag="mid")
            nc.gpsimd.tensor_tensor(_blk(mid[:], 48), s3[:, :, 1:49],
                                    s3[:, :, 0:48], Alu.add)
            nc.gpsimd.tensor_scalar(mid[:], mid[:], 0.5, None, Alu.mult)
            wm = pool.tile([P, NBLK * 48], dt.float32, tag="wm")
            nc.gpsimd.tensor_tensor(wm[:], rw_sh[:], mid[:], Alu.mult)
            Cin = pool.tile([P, NBLK * 48], dt.float32, tag="Cin")
            nc.vector.tensor_tensor_scan(Cin[:], mask48[:], rw_sh[:], 0.0,
                                         Alu.mult, Alu.add)
            Sin = pool.tile([P, NBLK * 48], dt.float32, tag="Sin")
            nc.vector.tensor_tensor_scan(Sin[:], mask48[:], wm[:], 0.0,
                                         Alu.mult, Alu.add)
            A = pool.tile([P, NBLK * 47], dt.float32, tag="A47")
            m3 = _blk(mid[:], 48)
            c3 = _blk(Cin[:], 48)
            sw3 = _blk(Sin[:], 48)
            rw3 = _blk(rw_sh[:], 48)
            A3 = _blk(A[:], 47)
            nc.gpsimd.tensor_tensor(A3, m3[:, :, 1:48], c3[:, :, 0:47], Alu.mult)
            nc.gpsimd.tensor_tensor(A3, A3, sw3[:, :, 0:47], Alu.subtract)
            nc.gpsimd.tensor_tensor(A3, A3, rw3[:, :, 1:48], Alu.mult)
            nc.vector.tensor_reduce(accs["p1"][:], A3, AX.XY, Alu.add)
            t2 = pool.tile([P, NBLK * 48], dt.float32, tag="t2d")
            nc.gpsimd.tensor_tensor(t2[:], rw_sh[:], rw_sh[:], Alu.mult)
            nc.gpsimd.tensor_tensor(t2[:], t2[:], ds[:], Alu.mult)
            nc.vector.tensor_reduce(accs["p2"][:], _blk(t2[:], 48), AX.XY, Alu.add)

        # ---------- inter loss (levels emitted concurrently) ----------
        inter_lvls = [l for l in (0, 1) if f"l{l}" in parts]
        wpool = ctx.enter_context(tc.tile_pool(name="work", bufs=1))
        lvl_pools = {l: ctx.enter_context(tc.tile_pool(name=f"lvl{l}", bufs=1))
                     for l in inter_lvls}
        tails = {}
        for lvl in inter_lvls:
            tails[lvl] = _emit_level(
                nc, tc, wpool, lvl_pools[lvl], lvl, ekeys, radios[lvl],
                emcs[lvl], epcs[lvl], aps[f"ps{lvl}"], aps[f"pw{lvl}"],
                accs["inter" if lvl == 0 else "inter1"], aps, neg05)
        if 0 in tails:
            tails[0]()

        # ---------- rgb ----------
        with tc.tile_pool(name="rgb", bufs=1) as pool:
            pdt = pool.tile([P, NBLK * 3], dt.float32, tag="pdt")
            gtt = pool.tile([P, NBLK * 3], dt.float32, tag="gtt")
            nc.sync.dma_start(_blk(pdt[:], 3),
                              aps["pd"].rearrange("(p b) c -> p b c", b=NBLK))
            nc.sync.dma_start(_blk(gtt[:], 3),
                              aps["gt"].rearrange("(p b) c -> p b c", b=NBLK))
            d = pool.tile([P, NBLK * 3], dt.float32, tag="rgbd")
            nc.vector.tensor_tensor(d[:], pdt[:], gtt[:], Alu.subtract)
            nc.vector.tensor_tensor(d[:], d[:], d[:], Alu.mult)
            nc.vector.tensor_reduce(accs["rgb"][:], d[:], AX.X, Alu.add)

        # ---------- hash ----------
        if "hash" in parts:
            hpool = ctx.enter_context(tc.tile_pool(name="hash", bufs=1))
            hv = {}
            for lvl in (0, 1):
                idx = hpool.tile([P, HCOLS], dt.int32, tag=f"hidx{lvl}",
                                 name=f"hidx{lvl}")
                src = aps[f"hi{lvl}"]
                nc.scalar.dma_start(idx[:], bass.AP(tensor=src.tensor,
                                                    offset=src.offset,
                                                    ap=[[HROW, P], [1, HCOLS]]))
                emb = hpool.tile([P, HCOLS * 2], dt.float32, tag=f"hemb{lvl}",
                                 name=f"hemb{lvl}")
                esrc = aps[f"he{lvl}"]
                nc.scalar.dma_start(emb[:], bass.AP(tensor=esrc.tensor,
                                                    offset=esrc.offset,
                                                    ap=[[HROW * 2, P],
                                                        [1, HCOLS * 2]]))
                hv[lvl] = dict(idx=idx, emb=emb)
            for lvl in (0, 1):
                v = hv[lvl]
                v["sq"] = hpool.tile([P, HCOLS * 2], dt.float32,
                                     tag=f"hsq{lvl}", name=f"hsq{lvl}")
                nc.scalar.activation(v["sq"][:], v["emb"][:], Act.Square)
                v["wv"] = hpool.tile([P, HCOLS], dt.float32, tag=f"hw{lvl}",
                                     name=f"hw{lvl}")
                sq3 = v["sq"][:].rearrange("p (n two) -> p n two", two=2)
                nc.vector.tensor_tensor(v["wv"][:], sq3[:, :, 0], sq3[:, :, 1],
                                        Alu.add)
            for lvl in (0, 1):
                v = hv[lvl]
                eq = hpool.tile([P, HCOLS], dt.float32, tag=f"heq{lvl}",
                                name=f"heq{lvl}")
                nc.vector.memset(eq[:, 0:1], 0.0)
                nc.vector.tensor_tensor(eq[:, 1:HCOLS], v["idx"][:, 1:HCOLS],
                                        v["idx"][:, 0:HCOLS - 1], Alu.is_equal)
                v["eq"] = eq
                v["me"] = hpool.tile([P, HROW], dt.float32, tag=f"hme{lvl}",
                                     name=f"hme{lvl}")
                nc.vector.tensor_scalar(v["me"][:],
                                        eq[:, HALO + 1:HALO + HROW + 1], -1.0,
                                        1.0, Alu.mult, Alu.add)
            for lvl in (0, 1):
                v = hv[lvl]
                v["S"] = hpool.tile([P, HCOLS], dt.float32, tag=f"hS{lvl}",
                                    name=f"hS{lvl}")
                nc.vector.tensor_tensor_scan(v["S"][:], v["eq"][:], v["wv"][:],
                                             0.0, Alu.mult, Alu.add)
            for lvl in (0, 1):
                v = hv[lvl]
                v["cc"] = hpool.tile([P, HCOLS], dt.float32, tag=f"hcc{lvl}",
                                     name=f"hcc{lvl}")
                nc.vector.tensor_tensor_scan(v["cc"][:], v["eq"][:], ones_h[:],
                                             0.0, Alu.mult, Alu.add)
            for lvl in (0, 1):
                nc.vector.reciprocal(hv[lvl]["cc"][:], hv[lvl]["cc"][:])
            for lvl in (0, 1):
                v = hv[lvl]
                nc.vector.tensor_tensor(v["S"][:], v["S"][:], v["cc"][:],
                                        Alu.mult)
            for lvl in (0, 1):
                v = hv[lvl]
                part = hpool.tile([P, 1], dt.float32, tag=f"hpart{lvl}",
                                  name=f"hpart{lvl}")
                nc.vector.scalar_tensor_tensor(v["S"][:, HALO:HALO + HROW],
                                               v["S"][:, HALO:HALO + HROW],
                                               1.0,
                                               v["me"][:],
                                               Alu.mult, Alu.mult,
                                               accum_out=part[:])
                if lvl == 0:
                    nc.vector.tensor_copy(accs["hash"][:], part[:])
                else:
                    nc.vector.tensor_tensor(accs["hash"][:], accs["hash"][:],
                                            part[:], Alu.add)

        if 1 in tails:
            tails[1]()

        # ---------- combine + output ----------
        with tc.tile_pool(name="fin", bufs=1) as pool:
            tot = pool.tile([P, 1], dt.float32, tag="tot")
            nc.vector.tensor_scalar(tot[:], accs["rgb"][:], W_RGB / (R * 3), None,
                                    Alu.mult)
            nc.vector.scalar_tensor_tensor(tot[:], accs["inter"][:], W_INTER,
                                           tot[:], Alu.mult, Alu.add)
            nc.vector.scalar_tensor_tensor(tot[:], accs["inter1"][:], W_INTER,
                                           tot[:], Alu.mult, Alu.add)
            nc.vector.scalar_tensor_tensor(tot[:], accs["p1"][:], W_DIST * 2.0 / R,
                                           tot[:], Alu.mult, Alu.add)
            nc.vector.scalar_tensor_tensor(tot[:], accs["p2"][:],
                                           W_DIST / (3.0 * R), tot[:],
                                           Alu.mult, Alu.add)
            nc.vector.scalar_tensor_tensor(tot[:], accs["hash"][:],
                                           W_HASH / (NUM_SEGMENTS * 2.0), tot[:],
                                           Alu.mult, Alu.add)
            import concourse.bass_isa as bass_isa
            red = pool.tile([P, 1], dt.float32, tag="red")
            nc.gpsimd.partition_all_reduce(red[:], tot[:], channels=P,
                                           reduce_op=bass_isa.ReduceOp.add)
            nc.sync.dma_start(out_ap, red[:][0:1, 0:1])


# ---------------- host side ----------------
_module_cache = {}


def _get_module():
    if "nc" not in _module_cache:
        _module_cache["nc"] = build_module()
    return _module_cache["nc"]


def shard_inputs(inputs):
    """Full inputs -> list of 8 per-core in_maps."""
    f32 = np.float32
    pd = np.ascontiguousarray(inputs["pd_rgbs"], f32)
    gt = np.ascontiguousarray(inputs["gt_rgbs"], f32)
    sd = np.ascontiguousarray(inputs["render_sdist"], f32)
    rw = np.ascontiguousarray(inputs["render_weights"], f32)
    ps0 = np.ascontiguousarray(inputs["prop_sdist_0"], f32)
    pw0 = np.ascontiguousarray(inputs["prop_weights_0"], f32)
    ps1 = np.ascontiguousarray(inputs["prop_sdist_1"], f32)
    pw1 = np.ascontiguousarray(inputs["prop_weights_1"], f32)
    hashes = {}
    for lvl in (0, 1):
        idx = np.asarray(inputs[f"enc_idx_{lvl}"]).astype(np.int32)
        emb = np.ascontiguousarray(inputs[f"enc_embds_{lvl}"], f32)
        idx_pad = np.full(M + 2 * HALO, -1, np.int32)
        idx_pad[HALO:HALO + M] = idx
        emb_pad = np.zeros((M + 2 * HALO, 2), f32)
        emb_pad[HALO:HALO + M] = emb
        hashes[lvl] = (idx_pad, emb_pad)

    consts = {}
    rep = lambda row: np.ascontiguousarray(np.tile(row, (P, 1)))
    for lvl, L in LVL.items():
        n2, LW = L["n2"], L["LW"]
        consts[f"c_iota16_l{lvl}"] = rep(np.tile(np.arange(n2, dtype=np.int16),
                                                 NBLK))
        consts[f"c_iota1_l{lvl}"] = rep(np.tile(
            np.arange(1, LW + 1, dtype=np.int16), NBLK))
        mi = np.ones(NBLK * LW, np.int16)
        mi[::LW] = 0
        consts[f"c_maski_l{lvl}"] = rep(mi)
        consts[f"c_maskf_l{lvl}"] = rep(mi.astype(np.float32))
    m48 = np.ones(NBLK * 48, np.float32)
    m48[::48] = 0.0
    consts["c_mask48"] = rep(m48)
    consts["c_ones"] = rep(np.ones(HCOLS, np.float32))

    in_maps = []
    for c in range(N_CORES):
        r0 = c * RPC
        lo = c * MPC
        im = {
            "pd": pd[r0:r0 + RPC], "gt": gt[r0:r0 + RPC],
            "sd": sd[r0:r0 + RPC], "rw": rw[r0:r0 + RPC],
            "ps0": ps0[r0:r0 + RPC], "pw0": pw0[r0:r0 + RPC],
            "ps1": ps1[r0:r0 + RPC], "pw1": pw1[r0:r0 + RPC],
        }
        for lvl in (0, 1):
            idx_pad, emb_pad = hashes[lvl]
            im[f"hi{lvl}"] = np.ascontiguousarray(idx_pad[lo:lo + HSLICE])
            im[f"he{lvl}"] = np.ascontiguousarray(
                emb_pad[lo:lo + HSLICE].reshape(-1))
        im.update(consts)
        in_maps.append(im)
    return in_maps


def kernel(**inputs) -> np.ndarray:
    nc = _get_module()
    in_maps = shard_inputs(inputs)
    res = run_bass_kernel_spmd(nc, in_maps, core_ids=list(range(N_CORES)))
    total = np.float64(0.0)
    for r in res.results:
        total += np.float64(r["out"][0, 0])
    return np.float32(total)
